# revision 26
# baseline (speedup 1.0000x reference)
"""LocalityEnhancedAttention Trainium2 kernel (8 NeuronCores, SPMD).

Sharding: core c handles batch b = c//2 and head-group g = c%2
(8 of 16 heads). Each core computes its partial output projection
(attn_heads @ wo_shard.T); host sums the two head-group partials per
batch and adds bo.

Device dataflow per core (S=2048, M=1024, local head-dims DH=512):
  - host pre-transposes inputs:  qT/kT/vT = x[b].T  [M, S]
  - projections: Q^T,K^T [DH, S] in [d, s] layout (N=1024 streams,
    bias folded into the PSUM->SBUF cast as a per-partition
    tensor_scalar add), V in [s, d] layout augmented with a ones
    column per head (softmax denominators ride along row 64 of the
    PV accumulation)
  - scores^T[kj, qi] = K^T.T @ Q^T per head, head pairs packed into
    PE row-halves via tile_position, banded local bias applied
    multiplicatively post-exp on the DVE, exp on ACT (scale=1/8)
  - PV: A^T_aug[65, qi] += V_aug[kj].T @ P^T[kj] accumulated in PSUM
  - normalization: reciprocal_approx_fast of the denominator row,
    gpsimd partition_broadcast, deferred in-place DVE muls
  - WO: out_partial[s, m] = sum_d A_norm^T.T @ woT (N=1024)

Scheduling: ACT (~266us of exp) is the bottleneck engine; emission is
ordered so it starts early (K proj chunk 0 -> Q proj ch0 -> first
scores) and never starves: WO and Q projections for later chunks are
spread as fillers inside the attention kj loops rather than bursts
between groups, and PV trails QK/exp by LAG tiles.
"""

import os
import sys
from collections import deque
from contextlib import ExitStack

import numpy as np

sys.path.insert(0, "/opt/trn_rl_repo")

import ml_dtypes

BF = ml_dtypes.bfloat16

import concourse.bass as bass
import concourse.mybir as mybir
import concourse.tile as tile
from concourse import bacc
from concourse.bass_utils import run_bass_kernel_spmd

F32 = mybir.dt.float32
BF16 = mybir.dt.bfloat16
EXP = mybir.ActivationFunctionType.Exp
MULT = mybir.AluOpType.mult
ADD = mybir.AluOpType.add
TS, DS = bass.ts, bass.ds

S = 2048
M = 1024
DH = 512        # head dims per core (8 heads x 64)
DK = 64
W = 16
NPT = 4         # head pairs per core
NCH = 4         # qi chunks of 512
NKJ = 16        # kj tiles of 128
LAG = 2         # PV trails QK/exp by this many kj tiles


def _emit(ctx, tc, io):
    nc = tc.nc

    const = ctx.enter_context(tc.tile_pool(name="const", bufs=1))
    qkvp = ctx.enter_context(tc.tile_pool(name="qkv", bufs=1))
    stream = ctx.enter_context(tc.tile_pool(name="stream", bufs=12))
    sqp = ctx.enter_context(tc.tile_pool(name="sqp", bufs=8))
    ptp = ctx.enter_context(tc.tile_pool(name="ptp", bufs=20))
    normp = ctx.enter_context(tc.tile_pool(name="normp", bufs=2))
    op = ctx.enter_context(tc.tile_pool(name="op", bufs=2))
    sps = ctx.enter_context(tc.tile_pool(name="sps", bufs=2, space="PSUM"))
    pvs = ctx.enter_context(tc.tile_pool(name="pvs", bufs=2, space="PSUM"))
    wops = ctx.enter_context(tc.tile_pool(name="wops", bufs=2, space="PSUM"))

    # ---------------- constants ----------------
    pat = const.tile([128, 320], BF16, tag="pat", name="pat")
    nc.sync.dma_start(pat[:], io["pat"])
    bqc = const.tile([128, NPT], F32, tag="bqc", name="bqc")
    nc.sync.dma_start(bqc[:], io["bqc"])
    bkc = const.tile([128, NPT], F32, tag="bkc", name="bkc")
    nc.sync.dma_start(bkc[:], io["bkc"])
    bv = const.tile([1, DH], BF16, tag="bv", name="bv")
    nc.sync.dma_start(bv[:], io["bv"])
    bvb = const.tile([128, DH], BF16, tag="bvb", name="bvb")
    nc.gpsimd.partition_broadcast(bvb[:], bv[:])

    woT_sb = [const.tile([128, M], BF16, tag=f"wo{i}", name=f"wo{i}") for i in range(NPT)]
    wq_sb = [const.tile([128, DH], BF16, tag=f"wq{k}", name=f"wq{k}") for k in range(8)]
    wk_sb = [const.tile([128, DH], BF16, tag=f"wk{k}", name=f"wk{k}") for k in range(8)]
    wv_sb = [const.tile([128, DH], BF16, tag=f"wv{k}", name=f"wv{k}") for k in range(8)]
    # K-path weights first (they gate the first matmuls); wv/woT last
    for k in range(8):
        nc.sync.dma_start(wk_sb[k][:], io["wkT"][TS(k, 128), :])

    # ---------------- persistent tiles ----------------
    qT_sb = [qkvp.tile([128, S], BF16, tag=f"q{i}", name=f"q{i}") for i in range(NPT)]
    kT_sb = [qkvp.tile([128, S], BF16, tag=f"k{i}", name=f"k{i}") for i in range(NPT)]
    v_sb = [qkvp.tile([128, 8 * 65], BF16, tag=f"v{i}", name=f"v{i}") for i in range(16)]
    a_sb = [qkvp.tile([128, S], BF16, tag=f"a{i}", name=f"a{i}") for i in range(NPT)]
    for st in range(16):
        vv = v_sb[st].rearrange("p (h e) -> p h e", e=65)
        nc.vector.memset(vv[:, :, 64:65], 1.0)

    patv = pat.rearrange("p (h w) -> p h w", h=2)

    # ---------------- projection emitters ----------------
    def load_half(x_name, k, half):
        t = stream.tile([128, 1024], BF16, tag="s", name="s")
        nc.sync.dma_start(t[:], io[x_name][TS(k, 128), TS(half, 1024)])
        return t

    def proj_k_pt(xh, sc, pt):
        # kT_sb[pt][:, sc*512:(sc+1)*512] from stream half xh (sc in half)
        ps = wops.tile([128, 512], F32, tag="ps", name="kps")
        for k in range(8):
            nc.tensor.matmul(
                ps[:], lhsT=wk_sb[k][:, TS(pt, 128)],
                rhs=xh[k][:, DS((sc % 2) * 512, 512)],
                start=(k == 0), stop=(k == 7), skip_group_check=True)
        nc.vector.tensor_scalar_add(
            kT_sb[pt][:, TS(sc, 512)], ps[:], bkc[:, DS(pt, 1)])

    def load_q(ch):
        xs = []
        for k in range(8):
            t = sqp.tile([128, 512], BF16, tag="sq", name="sq")
            nc.sync.dma_start(t[:], io["qT"][TS(k, 128), TS(ch, 512)])
            xs.append(t)
        return xs

    def proj_q_pt(xs, ch, pt):
        ps = wops.tile([128, 512], F32, tag="ps", name="qps")
        for k in range(8):
            nc.tensor.matmul(
                ps[:], lhsT=wq_sb[k][:, TS(pt, 128)], rhs=xs[k][:],
                start=(k == 0), stop=(k == 7), skip_group_check=True)
        nc.vector.tensor_scalar_add(
            qT_sb[pt][:, TS(ch, 512)], ps[:], bqc[:, DS(pt, 1)])

    def proj_v_st(xh, st):
        # v_sb[st] from stream half xh (st within that half)
        ps = wops.tile([128, DH], F32, tag="ps", name="vps")
        for k in range(8):
            nc.tensor.matmul(
                ps[:], lhsT=xh[k][:, TS(st % 8, 128)], rhs=wv_sb[k][:],
                start=(k == 0), stop=(k == 7), skip_group_check=True)
        vv = v_sb[st].rearrange("p (h e) -> p h e", e=65)
        ps3 = ps.rearrange("p (h e) -> p h e", e=64)
        bv3 = bvb.rearrange("p (h e) -> p h e", e=64)
        nc.vector.scalar_tensor_tensor(
            vv[:, :, 0:64], ps3[:, :, :], 1.0, bv3[:, :, :], MULT, ADD)

    # ---------------- attention emitters ----------------
    def qk_exp(ch, pt, kj):
        c0, kj0 = ch * 512, kj * 128
        sp = sps.tile([128, 1024], F32, tag="sp", name="sp")
        for h in (0, 1):
            nc.tensor.matmul(
                sp[:, DS(h * 512, 512)],
                lhsT=kT_sb[pt][DS(h * 64, 64), TS(kj, 128)],
                rhs=qT_sb[pt][DS(h * 64, 64), TS(ch, 512)],
                start=True, stop=True,
                tile_position=(h * 64, 0),
                skip_group_check=True)
        ptt = ptp.tile([128, 1024], BF16, tag="ptt", name="ptt")
        nc.scalar.activation(ptt[:], sp[:], EXP, scale=0.125)
        lo = max(kj0 - W, c0)
        hi = min(kj0 + 128 + W, c0 + 512)
        if lo < hi:
            pa = lo - (kj0 - W)
            pv3 = ptt.rearrange("p (h w) -> p h w", h=2)
            nc.vector.tensor_mul(
                pv3[:, :, DS(lo - c0, hi - lo)],
                pv3[:, :, DS(lo - c0, hi - lo)],
                patv[:, :, DS(pa, hi - lo)])
        return ptt

    def pv(pt, kj, ptt, pvt):
        for h in (0, 1):
            nc.tensor.matmul(
                pvt[h][:],
                lhsT=v_sb[kj][:, DS((pt * 2 + h) * 65, 65)],
                rhs=ptt[:, DS(h * 512, 512)],
                start=(kj == 0), stop=(kj == 15),
                skip_group_check=True)

    def finish_group(ch, pt, pvt):
        rbs = []
        for h in (0, 1):
            dt = normp.tile([1, 512], F32, tag="dt", name="dt", bufs=2)
            nc.vector.tensor_copy(dt[:], pvt[h][DS(64, 1), :])
            rf = normp.tile([1, 512], F32, tag="rf", name="rf", bufs=2)
            nc.vector.reciprocal_approx_fast(rf[:], dt[:])
            rb = normp.tile([64, 512], F32, tag="rb", name="rb", bufs=4)
            nc.gpsimd.partition_broadcast(rb[:], rf[:])
            rbs.append(rb)
        return (ch, pt, pvt, rbs)

    def norm_muls(pending):
        if pending is None:
            return
        ch, pt, pvt, rbs = pending
        for h in (0, 1):
            nc.vector.tensor_mul(
                a_sb[pt][DS(h * 64, 64), TS(ch, 512)],
                pvt[h][DS(0, 64), :], rbs[h][:])

    def wo_half(st, mt, ot):
        # half of one output row-block; keeps Tensor bursts ~1us and off
        # the scores (sps) ring so ACT never loses its cushion
        pso = wops.tile([128, 512], F32, tag="ps", name="pso")
        for pt in range(NPT):
            nc.tensor.matmul(
                pso[:],
                lhsT=a_sb[pt][:, TS(st, 128)],
                rhs=woT_sb[pt][:, TS(mt, 512)],
                start=(pt == 0), stop=(pt == 3),
                skip_group_check=True)
        nc.vector.tensor_copy(ot[:, TS(mt, 512)], pso[:])
        nc.sync.dma_start(io["out"][TS(st, 128), TS(mt, 512)], ot[:, TS(mt, 512)])

    def wo_st(st):
        ot = op.tile([128, 1024], F32, tag="ot", name="ot")
        wo_half(st, 0, ot)
        wo_half(st, 1, ot)

    fillers = deque()

    def group(ch, pt, pending, pre=(), pre_n=0):
        pvt = [pvs.tile([65, 512], F32, tag="pv", name="pv") for _ in (0, 1)]
        live = dict(enumerate(pre))
        for kj in range(NKJ + LAG):
            if kj < NKJ and kj >= pre_n:
                live[kj] = qk_exp(ch, pt, kj)
            if kj == 1:
                norm_muls(pending)
            if kj >= LAG:
                pv(pt, kj - LAG, live.pop(kj - LAG), pvt)
            if kj in (3, 5, 7, 9, 11, 13) and fillers:
                fillers.popleft()()
        return finish_group(ch, pt, pvt)

    def flat_attention(glist, pendings):
        # One continuous stream: QK/exp for glist groups in order, PV
        # lagging LAG slots globally so group boundaries have no PV-tail
        # burst in front of the next group's first scores.
        livemap = {}
        pvts = {}
        total = len(glist) * NKJ
        for t in range(total + LAG):
            if t < total:
                gi, kj = divmod(t, NKJ)
                ch, pt = glist[gi]
                livemap[(gi, kj)] = qk_exp(ch, pt, kj)
                if kj == 2:
                    norm_muls(pendings.pop(gi - 1, None))
                if kj in (3, 5, 7, 9, 11, 13) and fillers:
                    fillers.popleft()()
            u = t - LAG
            if u >= 0:
                gi, kj = divmod(u, NKJ)
                ch, pt = glist[gi]
                if kj == 0:
                    pvts[gi] = [pvs.tile([65, 512], F32, tag="pv", name="pv")
                                for _ in (0, 1)]
                pv(pt, kj, livemap.pop((gi, kj)), pvts[gi])
                if kj == NKJ - 1:
                    pendings[gi] = finish_group(ch, pt, pvts.pop(gi))
        return pendings[len(glist) - 1]

    # ---------------- main schedule ----------------
    xk0 = [load_half("kT", k, 0) for k in range(8)]
    xq0 = load_q(0)
    for k in range(8):
        nc.sync.dma_start(wq_sb[k][:], io["wqT"][TS(k, 128), :])
    for k in range(8):
        nc.sync.dma_start(wv_sb[k][:], io["wvT"][TS(k, 128), :])
    for i in range(NPT):
        nc.sync.dma_start(woT_sb[i][:], io["woT"][TS(i, 128), :])
    for pt in range(NPT):
        proj_k_pt(xk0, 0, pt)
    xk1 = [load_half("kT", k, 1) for k in range(8)]
    for pt in range(NPT):
        proj_q_pt(xq0, 0, pt)

    # group (0,0) scores/exp with the remaining K chunks woven in as a
    # wavefront (kj tiles only need their K chunk), then V overlapping
    live0 = []
    for kj in range(NKJ):
        live0.append(qk_exp(0, 0, kj))
        if kj < 12:
            sc = 1 + kj // 4
            xh = xk0 if sc < 2 else xk1
            proj_k_pt(xh, sc, kj % 4)
    xv0 = [load_half("vT", k, 0) for k in range(8)]
    for st in range(8):
        proj_v_st(xv0, st)
    xv1 = [load_half("vT", k, 1) for k in range(8)]

    # PV(0,0) with V half-1 pieces and a (0,1) exp prelude interleaved
    pvt0 = [pvs.tile([65, 512], F32, tag="pv", name="pv") for _ in (0, 1)]
    pre01 = []
    for j in range(8):
        if j < 4:
            proj_v_st(xv1, 8 + 2 * j)
            proj_v_st(xv1, 9 + 2 * j)
        pv(0, 2 * j, live0[2 * j], pvt0)
        pv(0, 2 * j + 1, live0[2 * j + 1], pvt0)
        pre01.append(qk_exp(0, 1, j))
    pending = finish_group(0, 0, pvt0)

    def push_wo(st):
        box = {}

        def fa():
            box["ot"] = op.tile([128, 1024], F32, tag="ot", name="ot")
            wo_half(st, 0, box["ot"])

        def fb():
            wo_half(st, 1, box["ot"])

        fillers.append(fa)
        fillers.append(fb)

    pending01 = group(0, 1, pending, pre=pre01, pre_n=8)

    # fillers for the steady stream: WO of finished chunks and Q
    # projections for upcoming chunks, in dependency-safe order
    xs1 = load_q(1)
    for p in range(NPT):
        fillers.append(lambda xs=xs1, p=p: proj_q_pt(xs, 1, p))
    glist = [(ch, pt) for ch in range(4) for pt in range(4)][2:]

    pendings = {-1: pending01}

    class FillFeeder:
        # push per-group fillers right as the stream enters each group
        def __init__(self):
            self.next_gi = 0

        def feed(self, gi):
            ch, pt = glist[gi]
            if pt == 0 and ch >= 1:
                for st in range((ch - 1) * 4, (ch - 1) * 4 + 4):
                    push_wo(st)
            if pt == 1 and 1 <= ch < 3:
                xs = load_q(ch + 1)
                for p in range(NPT):
                    fillers.append(
                        lambda xs=xs, ch=ch, p=p: proj_q_pt(xs, ch + 1, p))

    feeder = FillFeeder()
    livemap = {}
    pvts = {}
    total = len(glist) * NKJ
    for t in range(total + LAG):
        # PV and fillers ahead of QK: if QK stalls on the scores ring
        # (ACT momentarily behind), Tensor still has ready work queued
        u = t - LAG
        if u >= 0:
            gi, kj = divmod(u, NKJ)
            ch, pt = glist[gi]
            if kj == 0:
                pvts[gi] = [pvs.tile([65, 512], F32, tag="pv", name="pv")
                            for _ in (0, 1)]
            pv(pt, kj, livemap.pop((gi, kj)), pvts[gi])
            if kj == NKJ - 1:
                pendings[gi] = finish_group(ch, pt, pvts.pop(gi))
        if t < total:
            gi, kj = divmod(t, NKJ)
            if kj == 0:
                feeder.feed(gi)
            if kj == 2:
                # runs after this t's PV section, where finish_group of
                # gi-1 was just emitted (kj 15 + LAG == this t), and
                # before pvt(gi) is allocated at the next t's PV section
                norm_muls(pendings.pop(gi - 1, None))
            if kj in (3, 5, 7, 9, 11, 13) and fillers:
                fillers.popleft()()
            ch, pt = glist[gi]
            livemap[(gi, kj)] = qk_exp(ch, pt, kj)
    norm_muls(pendings[len(glist) - 1])
    while fillers:
        fillers.popleft()()
    for st in range(12, 16):
        wo_st(st)


_CACHE = {}


def _build():
    if "nc" in _CACHE:
        return _CACHE["nc"]
    nc = bacc.Bacc("TRN2", target_bir_lowering=False, debug=False)
    io = {}
    for name, shape in (
        ("qT", [M, S]), ("kT", [M, S]), ("vT", [M, S]),
        ("wqT", [M, DH]), ("wkT", [M, DH]), ("wvT", [M, DH]),
        ("woT", [DH, M]),
        ("bv", [1, DH]),
    ):
        io[name] = nc.dram_tensor(name, shape, BF16, kind="ExternalInput").ap()
    io["bqc"] = nc.dram_tensor("bqc", [128, NPT], F32, kind="ExternalInput").ap()
    io["bkc"] = nc.dram_tensor("bkc", [128, NPT], F32, kind="ExternalInput").ap()
    io["pat"] = nc.dram_tensor("pat", [128, 320], BF16, kind="ExternalInput").ap()
    io["out"] = nc.dram_tensor("out", [S, M], F32, kind="ExternalOutput").ap()
    with tile.TileContext(nc) as tc:
        with ExitStack() as ctx:
            _emit(ctx, tc, io)
    nc.compile()
    _CACHE["nc"] = nc
    return nc


def _bias_pattern(local_bias):
    # multiplicative band pattern: exp(2*b[qi-kj+W]) inside the band, 1.0
    # outside; duplicated side by side for the two heads of a pair tile.
    p = np.arange(128)[:, None]
    f = np.arange(160)[None, :]
    idx = f - p  # rel + W
    valid = (idx >= 0) & (idx <= 2 * W)
    b = np.asarray(local_bias, np.float64)
    pat = np.where(valid, np.exp(2.0 * b[np.clip(idx, 0, 2 * W)]), 1.0)
    pat2 = np.concatenate([pat, pat], axis=1)
    return np.ascontiguousarray(pat2).astype(BF)


def kernel(query, key, value, wq, bq, wk, bk, wv, bv, wo, bo, local_bias):
    query = np.asarray(query, np.float32)
    key = np.asarray(key, np.float32)
    value = np.asarray(value, np.float32)
    wq, wk, wv, wo = (np.asarray(x, np.float32) for x in (wq, wk, wv, wo))
    bq, bk, bv, bo = (np.asarray(x, np.float32) for x in (bq, bk, bv, bo))
    pat = _bias_pattern(local_bias)

    nc = _build()
    in_maps = []
    for c in range(8):
        b, g = c // 2, c % 2
        sl = slice(g * DH, (g + 1) * DH)
        in_maps.append({
            "qT": np.ascontiguousarray(query[b].T).astype(BF),
            "kT": np.ascontiguousarray(key[b].T).astype(BF),
            "vT": np.ascontiguousarray(value[b].T).astype(BF),
            "wqT": np.ascontiguousarray(wq[sl, :].T).astype(BF),
            "wkT": np.ascontiguousarray(wk[sl, :].T).astype(BF),
            "wvT": np.ascontiguousarray(wv[sl, :].T).astype(BF),
            "woT": np.ascontiguousarray(wo[:, sl].T).astype(BF),
            "bqc": np.ascontiguousarray(bq[sl].reshape(NPT, 128).T).astype(np.float32),
            "bkc": np.ascontiguousarray(bk[sl].reshape(NPT, 128).T).astype(np.float32),
            "bv": np.ascontiguousarray(bv[sl]).reshape(1, DH).astype(BF),
            "pat": pat,
        })
    res = run_bass_kernel_spmd(
        nc, in_maps, core_ids=list(range(8)),
        trace=bool(int(os.environ.get("KERNEL_TRACE", "0"))),
    )
    _CACHE["last_result"] = res
    outs = [r["out"] for r in res.results]
    out = np.stack([outs[2 * b] + outs[2 * b + 1] + bo for b in range(4)])
    return out.astype(np.float32)
